# revision 16
# baseline (speedup 1.0000x reference)
"""Trainium2 Bass kernel for FOAM embedding (GNN message passing).

Strategy (8 NeuronCores, SPMD, no collectives):
  - Edges are sorted by edge_src. Host partitions nodes into 8 contiguous
    ranges with balanced edge counts; each core owns its nodes' edges.
  - Within a core, nodes are packed greedily into "blocks" of <=128 edges
    and <=8 node slots. Each block's 128 edge slots sit on the 128 SBUF
    partitions.
  - The segment-sum over edges becomes one PE matmul per block:
        lhsT = Dij [128e x 128b]   (stationary)
        rhs  = S   [128e x 80]     S[e, l*10+m] = onehot(slot l) * Y[e, m]
    giving PSUM [128b x (slot, m)] = rhoi for up to 8 nodes at once.
  - Phase 3 contracts rhoi with the (row-permuted) Dense weights over the
    128 basis dim on the PE, then DVE mult + strided reduce for
    (xl*yl).sum(m).
  - Host reassembles the full [15000, 528] output (species enc columns are
    a pure table gather, done on host).
"""

import os
import sys

import numpy as np

for _p in ("/opt/trn_rl_repo", "/root/.axon_site/_ro/trn_rl_repo"):
    if os.path.isdir(_p) and _p not in sys.path:
        sys.path.insert(0, _p)

import ml_dtypes  # noqa: E402

# ---------------- problem constants (hardcoded per spec) ----------------
N_RADIAL = 8
N_SPEC = 16
ZMAX = 64
CUTOFF = 5.0
NCHAN = 128
NB = N_RADIAL * N_SPEC  # 128 basis
M9 = 9                  # real SH components up to l=2
M10 = 10                # padded (plane 9 is zero)

NCORES = 8
P = 128                 # edges per block == partitions
NSLOT = 8               # node slots per block
SCOLS = NSLOT * M10     # 80 moving columns per block
CH = 60                 # blocks per chunk
PSG = 6                 # blocks per PSUM scatter tile (6*80=480 <= 512)

BF16 = ml_dtypes.bfloat16

_COMPILED = {}
TRACE = False          # set True to capture an NTFF profile
LAST_RESULT = None     # BassKernelResults of the last kernel() call


# ======================= host-side preprocessing =======================

def _partition_and_pack(edge_src, n_nodes):
    """Split nodes into NCORES contiguous ranges (edge balanced), then pack
    nodes into blocks of <=P edges / <=NSLOT nodes per core.

    Returns per-core dicts with block structure.
    """
    N_NODES = n_nodes
    es = np.asarray(edge_src, dtype=np.int64)
    E = es.shape[0]
    deg = np.bincount(es, minlength=N_NODES)
    # node boundaries: node of the edge at each split point
    splits = [0]
    for c in range(1, NCORES):
        n = int(es[min((c * E) // NCORES, E - 1)])
        n = max(n, splits[-1])
        splits.append(n)
    splits.append(N_NODES)

    cores = []
    for c in range(NCORES):
        nlo, nhi = splits[c], splits[c + 1]
        blocks = []  # (node_start, n_nodes, edge_count)
        n = nlo
        while n < nhi:
            cnt = 0
            esum = 0
            while (
                n + cnt < nhi
                and cnt < NSLOT
                and esum + deg[n + cnt] <= P
            ):
                esum += deg[n + cnt]
                cnt += 1
            if cnt == 0:
                raise ValueError(
                    f"node {n} has degree {deg[n]} > {P}; unsupported"
                )
            blocks.append((n, cnt, esum))
            n += cnt
        cores.append({"nlo": nlo, "nhi": nhi, "blocks": blocks})
    return cores, deg


def _build_host_inputs(inputs, cores, deg, B, nchunk):
    """Build per-core DRAM input arrays in the device layout."""
    dist = np.asarray(inputs["distances"], np.float32)
    vec = np.asarray(inputs["vec"], np.float32)
    switch = np.asarray(inputs["switch"], np.float32)
    st = np.asarray(inputs["species_table"], np.float32)
    species = np.asarray(inputs["species"], np.int64)
    esrc = np.asarray(inputs["edge_src"], np.int64)
    edst = np.asarray(inputs["edge_dst"], np.int64)
    N_NODES = species.shape[0]

    senc_node = st[species]          # [N, 16]
    first_edge = np.searchsorted(esrc, np.arange(N_NODES + 1), side="left")

    per_core = []
    for c in range(NCORES):
        blocks = cores[c]["blocks"]
        nb = len(blocks)
        # per-edge-slot fill values (padded)
        edf = np.zeros((B, 5, P), np.float32)
        edf[:, 0, :] = 1.0                      # dist pad
        edf[:, 2, :] = 1.0                      # vx pad
        senc_e = np.zeros((B, P, N_SPEC), np.float32)
        oh = np.zeros((B, P, NSLOT), np.float32)
        slot_node = np.full((B * NSLOT,), -1, np.int64)

        for k, (n0, cnt, esum) in enumerate(blocks):
            e0 = first_edge[n0]
            e1 = first_edge[n0 + cnt]
            idx = np.arange(e0, e1)
            p = idx - e0
            edf[k, 0, p] = dist[idx]
            edf[k, 1, p] = switch[idx]
            edf[k, 2, p] = vec[idx, 0]
            edf[k, 3, p] = vec[idx, 1]
            edf[k, 4, p] = vec[idx, 2]
            senc_e[k, p, :] = senc_node[edst[idx]]
            loc = esrc[idx] - n0
            oh[k, p, loc] = 1.0
            slot_node[k * NSLOT: k * NSLOT + cnt] = np.arange(n0, n0 + cnt)

        # device layouts (chunk-major not needed: tensors sliced per chunk
        # along the free axis; keep [128, ...] with block dim inside)
        # edf_dev: [nchunk, 128, 5, CH]  (partition-contiguous runs of 5*CH)
        edf_dev = np.ascontiguousarray(
            edf.reshape(nchunk, CH, 5, P).transpose(0, 3, 2, 1)
        )  # [nchunk, 128, 5, CH]
        senc_dev = np.ascontiguousarray(
            senc_e.reshape(nchunk, CH, P, N_SPEC).transpose(0, 2, 1, 3)
        ).astype(BF16)  # [nchunk, 128, CH, 16]
        oh_dev = np.ascontiguousarray(
            oh.reshape(nchunk, CH, P, NSLOT).transpose(0, 2, 1, 3)
        ).astype(BF16)  # [nchunk, 128, CH, 8]

        per_core.append(
            {
                "edf": edf_dev.reshape(nchunk, P, 5 * CH),
                "senc": senc_dev.reshape(nchunk, P, CH * N_SPEC),
                "oh": oh_dev.reshape(nchunk, P, CH * NSLOT),
                "slot_node": slot_node,
                "nblocks": nb,
            }
        )
    return per_core


def _perm_w(W):
    """Permute Dense weight rows from rs-order (r*16+s) to (s*8+r) order."""
    W = np.asarray(W, np.float32)
    return np.ascontiguousarray(
        W.reshape(N_RADIAL, N_SPEC, -1).transpose(1, 0, 2).reshape(NB, -1)
    )


# ========================= device program =========================

def _build_program(nchunk):
    import concourse.bacc as bacc
    import concourse.mybir as mybir
    import concourse.tile as tile
    from concourse.alu_op_type import AluOpType as alu

    fp32 = mybir.dt.float32
    bf16 = mybir.dt.bfloat16

    B = nchunk * CH
    NS = NSLOT * B

    nc = bacc.Bacc("TRN2", target_bir_lowering=False, debug=False,
                   num_devices=NCORES)

    edf_d = nc.dram_tensor("edf", [nchunk, P, 5 * CH], fp32,
                           kind="ExternalInput")
    senc_d = nc.dram_tensor("senc", [nchunk, P, CH * N_SPEC], bf16,
                            kind="ExternalInput")
    oh_d = nc.dram_tensor("oh", [nchunk, P, CH * NSLOT], bf16,
                          kind="ExternalInput")
    wx_d = nc.dram_tensor("wx", [P, 3 * NCHAN], bf16, kind="ExternalInput")
    wy_d = nc.dram_tensor("wy", [P, 3 * NCHAN], bf16, kind="ExternalInput")
    rhoi0_d = nc.dram_tensor("rhoi0", [P, NS], bf16, kind="ExternalOutput")
    xy_d = nc.dram_tensor("xy", [P, 3 * NS], fp32, kind="ExternalOutput")

    s3 = 3.0 ** 0.5
    s5 = 5.0 ** 0.5
    s15 = 15.0 ** 0.5
    bess = (2.0 / CUTOFF) ** 0.5

    with tile.TileContext(nc) as tc:
        with (
            tc.tile_pool(name="const", bufs=1) as cpool,
            tc.tile_pool(name="chunk", bufs=2) as ckpool,
            tc.tile_pool(name="big", bufs=1) as bigpool,
            tc.tile_pool(name="ps_sc", bufs=4, space="PSUM") as pssc,
            tc.tile_pool(name="ps_xy", bufs=2, space="PSUM") as psxy,
        ):
            wx = cpool.tile([P, 3 * NCHAN], bf16, tag="wx")
            wy = cpool.tile([P, 3 * NCHAN], bf16, tag="wy")
            nc.sync.dma_start(out=wx[:], in_=wx_d[:])
            nc.sync.dma_start(out=wy[:], in_=wy_d[:])
            half_pi = cpool.tile([P, 1], fp32, tag="halfpi")
            nc.vector.memset(half_pi[:], float(np.pi / 2))

            rhoi_sb = bigpool.tile([P, M10 * NS], bf16, tag="rhoi")
            xy_sb = bigpool.tile([P, 3 * NS], fp32, tag="xysb")

            ncopy = 0
            for ci in range(nchunk):
                edf = ckpool.tile([P, 5 * CH], fp32, tag="edf")
                senc = ckpool.tile([P, CH * N_SPEC], bf16, tag="senc")
                oh = ckpool.tile([P, CH * NSLOT], bf16, tag="oh")
                nc.sync.dma_start(out=edf[:], in_=edf_d[ci])
                nc.sync.dma_start(out=senc[:], in_=senc_d[ci])
                nc.sync.dma_start(out=oh[:], in_=oh_d[ci])

                d_ap = edf[:, 0 * CH:1 * CH]
                sw_ap = edf[:, 1 * CH:2 * CH]
                v_ap = edf[:, 2 * CH:5 * CH]

                rinv = ckpool.tile([P, CH], fp32, tag="rinv")
                nc.vector.reciprocal(out=rinv[:], in_=d_ap)

                # radial: rb[n] = sin(c_{n+1} d), c_n = n*pi/CUTOFF.
                # ACT Sin is only accurate on [-4.18, 4.18]; theta = pi*d/5
                # stays within range for this data (d <= ~6), so build the
                # higher harmonics with the Chebyshev recurrence
                #   sin((n+1)t) = 2cos(t) sin(nt) - sin((n-1)t).
                rb = ckpool.tile([P, N_RADIAL * CH], fp32, tag="rb")
                cos2 = ckpool.tile([P, CH], fp32, tag="cos2")
                nc.scalar.activation(
                    out=rb[:, 0:CH], in_=d_ap,
                    func=mybir.ActivationFunctionType.Sin,
                    scale=float(np.pi / CUTOFF),
                )
                nc.scalar.activation(
                    out=cos2[:], in_=d_ap,
                    func=mybir.ActivationFunctionType.Sin,
                    scale=float(-np.pi / CUTOFF), bias=half_pi[:],
                )
                nc.vector.tensor_scalar(
                    out=cos2[:], in0=cos2[:], scalar1=2.0, scalar2=None,
                    op0=alu.mult,
                )
                nc.vector.tensor_tensor(
                    out=rb[:, CH:2 * CH], in0=cos2[:], in1=rb[:, 0:CH],
                    op=alu.mult)
                for n in range(2, N_RADIAL):
                    nc.vector.tensor_tensor(
                        out=rb[:, n * CH:(n + 1) * CH], in0=cos2[:],
                        in1=rb[:, (n - 1) * CH:n * CH], op=alu.mult)
                    nc.vector.tensor_tensor(
                        out=rb[:, n * CH:(n + 1) * CH],
                        in0=rb[:, n * CH:(n + 1) * CH],
                        in1=rb[:, (n - 2) * CH:(n - 1) * CH],
                        op=alu.subtract)
                swf = ckpool.tile([P, CH], fp32, tag="swf")
                nc.vector.scalar_tensor_tensor(
                    out=swf[:], in0=sw_ap, scalar=float(bess), in1=rinv[:],
                    op0=alu.mult, op1=alu.mult,
                )
                rbf = ckpool.tile([P, N_RADIAL * CH], bf16, tag="rbf")
                nc.vector.tensor_tensor(
                    out=rbf[:].rearrange("p (n c) -> p n c", n=N_RADIAL),
                    in0=rb[:].rearrange("p (n c) -> p n c", n=N_RADIAL),
                    in1=swf[:].unsqueeze(1).broadcast_to([P, N_RADIAL, CH]),
                    op=alu.mult,
                )

                # unit vectors
                u = ckpool.tile([P, 3 * CH], fp32, tag="u")
                nc.vector.tensor_tensor(
                    out=u[:].rearrange("p (t c) -> p t c", t=3),
                    in0=v_ap.rearrange("p (t c) -> p t c", t=3),
                    in1=rinv[:].unsqueeze(1).broadcast_to([P, 3, CH]),
                    op=alu.mult,
                )
                ux, uy, uz = (u[:, i * CH:(i + 1) * CH] for i in range(3))

                # Y planes [P, M10, CH] bf16; plane 9 stays zero
                Y = ckpool.tile([P, M10 * CH], bf16, tag="Y")
                nc.vector.memset(Y[:, 0:CH], 1.0)
                nc.vector.memset(Y[:, 9 * CH:10 * CH], 0.0)
                nc.vector.tensor_scalar(
                    out=Y[:, 1 * CH:4 * CH], in0=u[:],
                    scalar1=float(s3), scalar2=None, op0=alu.mult,
                )
                nc.vector.scalar_tensor_tensor(
                    out=Y[:, 4 * CH:5 * CH], in0=ux, scalar=float(s15),
                    in1=uy, op0=alu.mult, op1=alu.mult)
                nc.vector.scalar_tensor_tensor(
                    out=Y[:, 5 * CH:6 * CH], in0=uy, scalar=float(s15),
                    in1=uz, op0=alu.mult, op1=alu.mult)
                nc.vector.scalar_tensor_tensor(
                    out=Y[:, 7 * CH:8 * CH], in0=ux, scalar=float(s15),
                    in1=uz, op0=alu.mult, op1=alu.mult)
                zz = ckpool.tile([P, CH], fp32, tag="zz")
                nc.vector.tensor_tensor(out=zz[:], in0=uz, in1=uz,
                                        op=alu.mult)
                nc.vector.tensor_scalar(
                    out=Y[:, 6 * CH:7 * CH], in0=zz[:],
                    scalar1=float(1.5 * s5), scalar2=float(-0.5 * s5),
                    op0=alu.mult, op1=alu.add,
                )
                sdif = ckpool.tile([P, CH], fp32, tag="sdif")
                ssum = ckpool.tile([P, CH], fp32, tag="ssum")
                nc.vector.tensor_tensor(out=sdif[:], in0=ux, in1=uy,
                                        op=alu.subtract)
                nc.vector.tensor_tensor(out=ssum[:], in0=ux, in1=uy,
                                        op=alu.add)
                nc.vector.scalar_tensor_tensor(
                    out=Y[:, 8 * CH:9 * CH], in0=sdif[:],
                    scalar=float(0.5 * s15), in1=ssum[:],
                    op0=alu.mult, op1=alu.mult)

                # S[p, blk, l*10+m] = oh[p, blk, l] * Y[p, m, blk]
                S = ckpool.tile([P, CH * SCOLS], bf16, tag="S")
                y3 = Y[:].rearrange("p (m c) -> p c m", m=M10)  # dims (blk, m)
                nc.gpsimd.tensor_tensor(
                    out=S[:].rearrange("p (c l m) -> p c l m", l=NSLOT, m=M10),
                    in0=oh[:].rearrange("p (c l) -> p c l", l=NSLOT)
                        .unsqueeze(3).broadcast_to([P, CH, NSLOT, M10]),
                    in1=y3.unsqueeze(2).broadcast_to([P, CH, NSLOT, M10]),
                    op=alu.mult,
                )

                # Dij[p, blk, s*8+r] = senc[p, blk, s] * rbf[p, r, blk]
                Dij = ckpool.tile([P, CH * NB], bf16, tag="Dij")
                nc.vector.tensor_tensor(
                    out=Dij[:].rearrange("p (c s r) -> p c s r",
                                         s=N_SPEC, r=N_RADIAL),
                    in0=senc[:].rearrange("p (c s) -> p c s", s=N_SPEC)
                        .unsqueeze(3).broadcast_to([P, CH, N_SPEC, N_RADIAL]),
                    in1=rbf[:].rearrange("p (r c) -> p c r", r=N_RADIAL)
                        .unsqueeze(2).broadcast_to([P, CH, N_SPEC, N_RADIAL]),
                    op=alu.mult,
                )

                # scatter matmuls: PSG blocks per PSUM tile, then one copy
                for g in range(CH // PSG):
                    pst = pssc.tile([P, PSG * SCOLS], fp32, tag="psc")
                    for j in range(PSG):
                        k = g * PSG + j
                        nc.tensor.matmul(
                            out=pst[:, j * SCOLS:(j + 1) * SCOLS],
                            lhsT=Dij[:, k * NB:(k + 1) * NB],
                            rhs=S[:, k * SCOLS:(k + 1) * SCOLS],
                            start=True, stop=True,
                        )
                    # copy PSUM -> rhoi_sb (skip m=9), cast to bf16
                    slot0 = (ci * CH + g * PSG) * NSLOT
                    src = pst[:].rearrange(
                        "p (b l m) -> p b l m", b=PSG, l=NSLOT)[:, :, :, 0:M9]
                    dst = rhoi_sb[:].rearrange(
                        "p (m s) -> p m s", m=M10)[
                        :, 0:M9, slot0:slot0 + PSG * NSLOT].rearrange(
                        "p m (b l) -> p b l m", b=PSG)
                    if ncopy % 2 == 0:
                        nc.scalar.copy(out=dst, in_=src)
                    else:
                        nc.vector.tensor_copy(out=dst, in_=src)
                    ncopy += 1

            # ---------------- phase 3 ----------------
            for l in range(3):
                mg = 2 * l + 1
                m0 = l * l
                sc = 480 // mg  # slots per chunk -> cols = sc*mg <= 480
                wxl = wx[:, l * NCHAN:(l + 1) * NCHAN]
                wyl = wy[:, l * NCHAN:(l + 1) * NCHAN]
                nslices = (NS + sc - 1) // sc
                for t in range(nslices):
                    s0 = t * sc
                    ssz = min(sc, NS - s0)
                    cols = ssz * mg
                    mov = rhoi_sb[:].rearrange("p (m s) -> p m s", m=M10)[
                        :, m0:m0 + mg, s0:s0 + ssz].rearrange(
                        "p m s -> p s m")
                    xps = psxy.tile([P, 480], fp32, tag="xps")
                    yps = psxy.tile([P, 480], fp32, tag="yps")
                    nc.tensor.matmul(out=xps[:, 0:cols], lhsT=wxl, rhs=mov,
                                     start=True, stop=True)
                    nc.tensor.matmul(out=yps[:, 0:cols], lhsT=wyl, rhs=mov,
                                     start=True, stop=True)
                    xsb = ckpool.tile([P, 480], fp32, tag="xsb")
                    nc.scalar.copy(out=xsb[:, 0:cols], in_=xps[:, 0:cols])
                    if mg == 1:
                        nc.vector.tensor_tensor(
                            out=xy_sb[:, l * NS + s0:l * NS + s0 + ssz],
                            in0=xsb[:, 0:cols], in1=yps[:, 0:cols],
                            op=alu.mult)
                    else:
                        txy = ckpool.tile([P, 480], fp32, tag="txy")
                        nc.vector.tensor_tensor(
                            out=txy[:, 0:cols], in0=xsb[:, 0:cols],
                            in1=yps[:, 0:cols], op=alu.mult)
                        nc.vector.tensor_reduce(
                            out=xy_sb[:, l * NS + s0:l * NS + s0 + ssz],
                            in_=txy[:, 0:cols].rearrange(
                                "p (s m) -> p s m", m=mg),
                            axis=mybir.AxisListType.X, op=alu.add,
                        )

            nc.sync.dma_start(out=rhoi0_d[:], in_=rhoi_sb[:, 0:NS])
            nc.sync.dma_start(out=xy_d[:], in_=xy_sb[:])

    nc.finalize()
    return nc


# ============================ entry point ============================

def kernel(**inputs):
    from concourse.bass_utils import run_bass_kernel_spmd

    species = np.asarray(inputs["species"], np.int64)
    N_NODES = species.shape[0]
    cores, deg = _partition_and_pack(np.asarray(inputs["edge_src"]), N_NODES)
    maxb = max(len(c["blocks"]) for c in cores)
    nchunk = (maxb + CH - 1) // CH
    B = nchunk * CH
    NS = NSLOT * B

    per_core = _build_host_inputs(inputs, cores, deg, B, nchunk)

    wx = np.empty((P, 3 * NCHAN), np.float32)
    wy = np.empty((P, 3 * NCHAN), np.float32)
    for l, key in enumerate(("W0", "W1", "W2")):
        Wp = _perm_w(inputs[key])
        wx[:, l * NCHAN:(l + 1) * NCHAN] = Wp[:, :NCHAN]
        wy[:, l * NCHAN:(l + 1) * NCHAN] = (
            Wp[:, NCHAN:] / np.sqrt(2 * l + 1.0))
    wx = wx.astype(BF16)
    wy = wy.astype(BF16)

    key = nchunk
    if key not in _COMPILED:
        _COMPILED[key] = _build_program(nchunk)
    nc = _COMPILED[key]

    in_maps = [
        {
            "edf": pc["edf"],
            "senc": pc["senc"],
            "oh": pc["oh"],
            "wx": wx,
            "wy": wy,
        }
        for pc in per_core
    ]
    res = run_bass_kernel_spmd(nc, in_maps, list(range(NCORES)),
                               trace=TRACE)
    global LAST_RESULT
    LAST_RESULT = res

    # ---------------- host assembly ----------------
    st = np.asarray(inputs["species_table"], np.float32)
    out = np.zeros((N_NODES, N_SPEC + NB + 3 * NCHAN), np.float32)
    out[:, :N_SPEC] = st[species]

    # device basis row of original index rs = r*16+s is dev = s*8+r
    r = np.arange(NB) // N_SPEC
    s = np.arange(NB) % N_SPEC
    dev_of_rs = s * N_RADIAL + r

    for c in range(NCORES):
        sn = per_core[c]["slot_node"]
        valid = sn >= 0
        nodes = sn[valid]
        slots = np.nonzero(valid)[0]
        r0 = np.asarray(res.results[c]["rhoi0"], np.float32)  # [128, NS]
        xy = res.results[c]["xy"]  # [128, 3*NS]
        out[nodes, N_SPEC:N_SPEC + NB] = r0[dev_of_rs][:, slots].T
        for l in range(3):
            out[nodes, N_SPEC + NB + l * NCHAN:N_SPEC + NB + (l + 1) * NCHAN] = (
                xy[:, l * NS + slots].T)
    return out


# revision 29
# speedup vs baseline: 1.6360x; 1.6360x over previous
"""Trainium2 Bass kernel for FOAM embedding (GNN message passing).

Strategy (8 NeuronCores, SPMD, no collectives):
  - Edges are sorted by edge_src. Host partitions nodes into 8 contiguous
    ranges with balanced edge counts; each core owns its nodes' edges.
  - Within a core, nodes are packed greedily into "blocks" of <=128 edges
    and <=8 node slots. Each block's 128 edge slots sit on the 128 SBUF
    partitions.
  - The segment-sum over edges becomes one PE matmul per block:
        lhsT = Dij [128e x 128b]   (stationary)
        rhs  = S   [128e x 80]     S[e, l*10+m] = onehot(slot l) * Y[e, m]
    giving PSUM [128b x (slot, m)] = rhoi for up to 8 nodes at once.
  - Phase 3 contracts rhoi with the (row-permuted) Dense weights over the
    128 basis dim on the PE, then DVE mult + strided reduce for
    (xl*yl).sum(m).
  - Host reassembles the full [15000, 528] output (species enc columns are
    a pure table gather, done on host).
"""

import os
import sys

import numpy as np

for _p in ("/opt/trn_rl_repo", "/root/.axon_site/_ro/trn_rl_repo"):
    if os.path.isdir(_p) and _p not in sys.path:
        sys.path.insert(0, _p)

import ml_dtypes  # noqa: E402

# ---------------- problem constants (hardcoded per spec) ----------------
N_RADIAL = 8
N_SPEC = 16
ZMAX = 64
CUTOFF = 5.0
NCHAN = 128
NB = N_RADIAL * N_SPEC  # 128 basis
M9 = 9                  # real SH components up to l=2
M10 = 10                # padded (plane 9 is zero)

NCORES = 8
P = 128                 # edges per block == partitions
NSLOT = 7               # node slots per block
SCOLS = NSLOT * M10     # 70 moving columns per block
CH = 42                 # blocks per chunk
PSG = 7                 # blocks per PSUM scatter tile (7*70=490 <= 512)

BF16 = ml_dtypes.bfloat16

_COMPILED = {}
TRACE = False          # set True to capture an NTFF profile
LAST_RESULT = None     # BassKernelResults of the last kernel() call


# ======================= host-side preprocessing =======================

def _partition_and_pack(edge_src, n_nodes):
    """Split nodes into NCORES contiguous ranges (edge balanced), then pack
    nodes into blocks of <=P edges / <=NSLOT nodes per core.

    Returns per-core dicts with block structure.
    """
    N_NODES = n_nodes
    es = np.asarray(edge_src, dtype=np.int64)
    E = es.shape[0]
    deg = np.bincount(es, minlength=N_NODES)
    # node boundaries: node of the edge at each split point
    splits = [0]
    for c in range(1, NCORES):
        n = int(es[min((c * E) // NCORES, E - 1)])
        n = max(n, splits[-1])
        splits.append(n)
    splits.append(N_NODES)

    cores = []
    for c in range(NCORES):
        nlo, nhi = splits[c], splits[c + 1]
        blocks = []  # (node_start, n_nodes, edge_count)
        n = nlo
        while n < nhi:
            cnt = 0
            esum = 0
            while (
                n + cnt < nhi
                and cnt < NSLOT
                and esum + deg[n + cnt] <= P
            ):
                esum += deg[n + cnt]
                cnt += 1
            if cnt == 0:
                raise ValueError(
                    f"node {n} has degree {deg[n]} > {P}; unsupported"
                )
            blocks.append((n, cnt, esum))
            n += cnt
        cores.append({"nlo": nlo, "nhi": nhi, "blocks": blocks})
    return cores, deg


def _build_host_inputs(inputs, cores, deg, B, nchunk):
    """Build per-core DRAM input arrays in the device layout."""
    dist = np.asarray(inputs["distances"], np.float32)
    vec = np.asarray(inputs["vec"], np.float32)
    switch = np.asarray(inputs["switch"], np.float32)
    st = np.asarray(inputs["species_table"], np.float32)
    species = np.asarray(inputs["species"], np.int64)
    esrc = np.asarray(inputs["edge_src"], np.int64)
    edst = np.asarray(inputs["edge_dst"], np.int64)
    N_NODES = species.shape[0]

    senc_node = st[species]          # [N, 16]
    first_edge = np.searchsorted(esrc, np.arange(N_NODES + 1), side="left")

    per_core = []
    for c in range(NCORES):
        blocks = cores[c]["blocks"]
        nb = len(blocks)
        # per-edge-slot fill values (padded)
        edf = np.zeros((B, 5, P), np.float32)
        edf[:, 0, :] = 1.0                      # dist pad
        edf[:, 2, :] = 1.0                      # vx pad
        senc_e = np.zeros((B, P, N_SPEC), np.float32)
        oh = np.zeros((B, P, NSLOT), np.float32)
        slot_node = np.full((B * NSLOT,), -1, np.int64)

        for k, (n0, cnt, esum) in enumerate(blocks):
            e0 = first_edge[n0]
            e1 = first_edge[n0 + cnt]
            idx = np.arange(e0, e1)
            p = idx - e0
            edf[k, 0, p] = dist[idx]
            edf[k, 1, p] = switch[idx]
            edf[k, 2, p] = vec[idx, 0]
            edf[k, 3, p] = vec[idx, 1]
            edf[k, 4, p] = vec[idx, 2]
            senc_e[k, p, :] = senc_node[edst[idx]]
            loc = esrc[idx] - n0
            oh[k, p, loc] = 1.0
            slot_node[k * NSLOT: k * NSLOT + cnt] = np.arange(n0, n0 + cnt)

        # device layouts: [nchunk, 128, ...] per-chunk contiguous.
        edf_dev = np.ascontiguousarray(
            edf.reshape(nchunk, CH, 5, P).transpose(0, 3, 2, 1)
        )  # [nchunk, 128, 5, CH]
        # senc_rep[p, c, s, r] = senc[p, c, s]  (packable TT operand)
        senc_dev = np.ascontiguousarray(
            np.repeat(
                senc_e.reshape(nchunk, CH, P, N_SPEC).transpose(0, 2, 1, 3),
                N_RADIAL, axis=3,
            )
        ).astype(BF16)  # [nchunk, 128, CH, 16*8]
        # oh_rep[p, c, l, m] = oh[p, c, l]
        oh_dev = np.ascontiguousarray(
            np.repeat(
                oh.reshape(nchunk, CH, P, NSLOT).transpose(0, 2, 1, 3),
                M10, axis=3,
            )
        ).astype(BF16)  # [nchunk, 128, CH, 7*10]

        per_core.append(
            {
                "edf": edf_dev.reshape(nchunk, P, 5 * CH),
                "senc": senc_dev.reshape(nchunk, P, CH * NB),
                "oh": oh_dev.reshape(nchunk, P, CH * SCOLS),
                "slot_node": slot_node,
                "nblocks": nb,
            }
        )
    return per_core


def _perm_w(W):
    """Permute Dense weight rows from rs-order (r*16+s) to (s*8+r) order."""
    W = np.asarray(W, np.float32)
    return np.ascontiguousarray(
        W.reshape(N_RADIAL, N_SPEC, -1).transpose(1, 0, 2).reshape(NB, -1)
    )


# ========================= device program =========================

def _build_program(nchunk):
    import concourse.bacc as bacc
    import concourse.mybir as mybir
    import concourse.tile as tile
    from concourse.alu_op_type import AluOpType as alu

    fp32 = mybir.dt.float32
    bf16 = mybir.dt.bfloat16

    B = nchunk * CH
    NS = NSLOT * B

    nc = bacc.Bacc("TRN2", target_bir_lowering=False, debug=False,
                   num_devices=NCORES)

    edf_d = nc.dram_tensor("edf", [nchunk, P, 5 * CH], fp32,
                           kind="ExternalInput")
    senc_d = nc.dram_tensor("senc", [nchunk, P, CH * NB], bf16,
                            kind="ExternalInput")
    oh_d = nc.dram_tensor("oh", [nchunk, P, CH * SCOLS], bf16,
                          kind="ExternalInput")
    wx_d = nc.dram_tensor("wx", [P, 3 * NCHAN], bf16, kind="ExternalInput")
    wy_d = nc.dram_tensor("wy", [P, 3 * NCHAN], bf16, kind="ExternalInput")
    rhoi0_d = nc.dram_tensor("rhoi0", [P, NS], bf16, kind="ExternalOutput")
    xy_d = nc.dram_tensor("xy", [P, 3 * NS], fp32, kind="ExternalOutput")

    s3 = 3.0 ** 0.5
    s5 = 5.0 ** 0.5
    s15 = 15.0 ** 0.5
    bess = (2.0 / CUTOFF) ** 0.5

    with tile.TileContext(nc) as tc:
        with (
            tc.tile_pool(name="const", bufs=1) as cpool,
            tc.tile_pool(name="chunk", bufs=2) as ckpool,
            tc.tile_pool(name="big", bufs=1) as bigpool,
            tc.tile_pool(name="ps_sc", bufs=4, space="PSUM") as pssc,
            tc.tile_pool(name="ps_xy", bufs=2, space="PSUM") as psxy,
        ):
            wx = cpool.tile([P, 3 * NCHAN], bf16, tag="wx")
            wy = cpool.tile([P, 3 * NCHAN], bf16, tag="wy")
            nc.sync.dma_start(out=wx[:], in_=wx_d[:])
            nc.sync.dma_start(out=wy[:], in_=wy_d[:])
            half_pi = cpool.tile([P, 1], fp32, tag="halfpi")
            nc.vector.memset(half_pi[:], float(np.pi / 2))

            rhoi_sb = bigpool.tile([P, M10 * NS], bf16, tag="rhoi")

            ncopy = 0
            for ci in range(nchunk):
                edf = ckpool.tile([P, 5 * CH], fp32, tag="edf")
                senc = ckpool.tile([P, CH * NB], bf16, tag="senc")
                oh = ckpool.tile([P, CH * SCOLS], bf16, tag="oh")
                nc.sync.dma_start(out=edf[:], in_=edf_d[ci])
                nc.sync.dma_start(out=senc[:], in_=senc_d[ci])
                nc.sync.dma_start(out=oh[:], in_=oh_d[ci])

                d_ap = edf[:, 0 * CH:1 * CH]
                sw_ap = edf[:, 1 * CH:2 * CH]
                v_ap = edf[:, 2 * CH:5 * CH]

                rinv = ckpool.tile([P, CH], fp32, tag="rinv")
                nc.vector.reciprocal(out=rinv[:], in_=d_ap)

                # radial: rb[n] = sin(c_{n+1} d), c_n = n*pi/CUTOFF.
                # ACT Sin is only accurate on [-4.18, 4.18]; theta = pi*d/5
                # stays within range for this data (d <= ~6), so build the
                # higher harmonics with the Chebyshev recurrence
                #   sin((n+1)t) = 2cos(t) sin(nt) - sin((n-1)t).
                rb = ckpool.tile([P, N_RADIAL * CH], fp32, tag="rb")
                cos2 = ckpool.tile([P, CH], fp32, tag="cos2")
                nc.scalar.activation(
                    out=rb[:, 0:CH], in_=d_ap,
                    func=mybir.ActivationFunctionType.Sin,
                    scale=float(np.pi / CUTOFF),
                )
                nc.scalar.activation(
                    out=cos2[:], in_=d_ap,
                    func=mybir.ActivationFunctionType.Sin,
                    scale=float(-np.pi / CUTOFF), bias=half_pi[:],
                )
                nc.vector.tensor_scalar(
                    out=cos2[:], in0=cos2[:], scalar1=2.0, scalar2=None,
                    op0=alu.mult,
                )
                nc.vector.tensor_tensor(
                    out=rb[:, CH:2 * CH], in0=cos2[:], in1=rb[:, 0:CH],
                    op=alu.mult)
                for n in range(2, N_RADIAL):
                    nc.vector.tensor_tensor(
                        out=rb[:, n * CH:(n + 1) * CH], in0=cos2[:],
                        in1=rb[:, (n - 1) * CH:n * CH], op=alu.mult)
                    nc.vector.tensor_tensor(
                        out=rb[:, n * CH:(n + 1) * CH],
                        in0=rb[:, n * CH:(n + 1) * CH],
                        in1=rb[:, (n - 2) * CH:(n - 1) * CH],
                        op=alu.subtract)
                swf = ckpool.tile([P, CH], fp32, tag="swf")
                nc.vector.scalar_tensor_tensor(
                    out=swf[:], in0=sw_ap, scalar=float(bess), in1=rinv[:],
                    op0=alu.mult, op1=alu.mult,
                )
                # rbf laid out r-innermost: [P, CH, 8]
                rbf = ckpool.tile([P, CH * N_RADIAL], bf16, tag="rbf")
                nc.vector.tensor_tensor(
                    out=rbf[:].rearrange("p (c n) -> p c n", n=N_RADIAL),
                    in0=rb[:].rearrange("p (n c) -> p n c", n=N_RADIAL)
                        .transpose([0, 2, 1]),
                    in1=swf[:].unsqueeze(2).broadcast_to([P, CH, N_RADIAL]),
                    op=alu.mult,
                )

                # unit vectors
                u = ckpool.tile([P, 3 * CH], fp32, tag="u")
                nc.vector.tensor_tensor(
                    out=u[:].rearrange("p (t c) -> p t c", t=3),
                    in0=v_ap.rearrange("p (t c) -> p t c", t=3),
                    in1=rinv[:].unsqueeze(1).broadcast_to([P, 3, CH]),
                    op=alu.mult,
                )
                ux, uy, uz = (u[:, i * CH:(i + 1) * CH] for i in range(3))

                # Y planes [P, M10, CH] bf16; plane 9 stays zero
                Y = ckpool.tile([P, M10 * CH], bf16, tag="Y")
                nc.vector.memset(Y[:, 0:CH], 1.0)
                nc.vector.memset(Y[:, 9 * CH:10 * CH], 0.0)
                nc.vector.tensor_scalar(
                    out=Y[:, 1 * CH:4 * CH], in0=u[:],
                    scalar1=float(s3), scalar2=None, op0=alu.mult,
                )
                nc.vector.scalar_tensor_tensor(
                    out=Y[:, 4 * CH:5 * CH], in0=ux, scalar=float(s15),
                    in1=uy, op0=alu.mult, op1=alu.mult)
                nc.vector.scalar_tensor_tensor(
                    out=Y[:, 5 * CH:6 * CH], in0=uy, scalar=float(s15),
                    in1=uz, op0=alu.mult, op1=alu.mult)
                nc.vector.scalar_tensor_tensor(
                    out=Y[:, 7 * CH:8 * CH], in0=ux, scalar=float(s15),
                    in1=uz, op0=alu.mult, op1=alu.mult)
                zz = ckpool.tile([P, CH], fp32, tag="zz")
                nc.vector.tensor_tensor(out=zz[:], in0=uz, in1=uz,
                                        op=alu.mult)
                nc.vector.tensor_scalar(
                    out=Y[:, 6 * CH:7 * CH], in0=zz[:],
                    scalar1=float(1.5 * s5), scalar2=float(-0.5 * s5),
                    op0=alu.mult, op1=alu.add,
                )
                sdif = ckpool.tile([P, CH], fp32, tag="sdif")
                ssum = ckpool.tile([P, CH], fp32, tag="ssum")
                nc.vector.tensor_tensor(out=sdif[:], in0=ux, in1=uy,
                                        op=alu.subtract)
                nc.vector.tensor_tensor(out=ssum[:], in0=ux, in1=uy,
                                        op=alu.add)
                nc.vector.scalar_tensor_tensor(
                    out=Y[:, 8 * CH:9 * CH], in0=sdif[:],
                    scalar=float(0.5 * s15), in1=ssum[:],
                    op0=alu.mult, op1=alu.mult)

                # S[p, blk, l*10+m] = oh_rep[p, blk, l, m] * Y[p, m, blk]
                # in0 contiguous bf16, in1 innermost step-1 -> 2x mode
                S = ckpool.tile([P, CH * SCOLS], bf16, tag="S")
                y3 = Y[:].rearrange("p (m c) -> p c m", m=M10)  # dims (blk, m)
                nc.vector.tensor_tensor(
                    out=S[:].rearrange("p (c l m) -> p c l m", l=NSLOT, m=M10),
                    in0=oh[:].rearrange("p (c l m) -> p c l m",
                                        l=NSLOT, m=M10),
                    in1=y3.unsqueeze(2).broadcast_to([P, CH, NSLOT, M10]),
                    op=alu.mult,
                )

                # Dij[p, blk, s*8+r] = senc_rep[p, blk, s, r] * rbf[p, blk, r]
                Dij = ckpool.tile([P, CH * NB], bf16, tag="Dij")
                nc.vector.tensor_tensor(
                    out=Dij[:].rearrange("p (c s r) -> p c s r",
                                         s=N_SPEC, r=N_RADIAL),
                    in0=senc[:].rearrange("p (c s r) -> p c s r",
                                          s=N_SPEC, r=N_RADIAL),
                    in1=rbf[:].rearrange("p (c r) -> p c r", r=N_RADIAL)
                        .unsqueeze(2).broadcast_to([P, CH, N_SPEC, N_RADIAL]),
                    op=alu.mult,
                )

                # scatter matmuls: PSG blocks per PSUM tile, then one
                # fully-contiguous copy into slot-major rhoi_sb
                # (rhoi_sb col = slot * 10 + m).
                for g in range(CH // PSG):
                    pst = pssc.tile([P, PSG * SCOLS], fp32, tag="psc")
                    for j in range(PSG):
                        k = g * PSG + j
                        nc.tensor.matmul(
                            out=pst[:, j * SCOLS:(j + 1) * SCOLS],
                            lhsT=Dij[:, k * NB:(k + 1) * NB],
                            rhs=S[:, k * SCOLS:(k + 1) * SCOLS],
                            start=True, stop=True,
                        )
                    col0 = (ci * CH + g * PSG) * NSLOT * M10
                    dst = rhoi_sb[:, col0:col0 + PSG * SCOLS]
                    if ncopy % 2 == 0:
                        nc.scalar.copy(out=dst, in_=pst[:])
                    else:
                        nc.vector.tensor_copy(out=dst, in_=pst[:])
                    ncopy += 1

            # ---------------- phase 3 ----------------
            # rhoi_sb is slot-major: col = slot*10 + m.  The l-group of a
            # slot is the contiguous m-run [l*l, (l+1)*(l+1)).
            for l in range(3):
                mg = 2 * l + 1
                m0 = l * l
                sc = 480 // mg  # slots per chunk -> cols = sc*mg <= 480
                wxl = wx[:, l * NCHAN:(l + 1) * NCHAN]
                wyl = wy[:, l * NCHAN:(l + 1) * NCHAN]
                nslices = (NS + sc - 1) // sc
                for t in range(nslices):
                    s0 = t * sc
                    ssz = min(sc, NS - s0)
                    cols = ssz * mg
                    mov = rhoi_sb[:].rearrange("p (s m) -> p s m", m=M10)[
                        :, s0:s0 + ssz, m0:m0 + mg]
                    xps = psxy.tile([P, 480], fp32, tag="xps")
                    yps = psxy.tile([P, 480], fp32, tag="yps")
                    nc.tensor.matmul(out=xps[:, 0:cols], lhsT=wxl, rhs=mov,
                                     start=True, stop=True)
                    nc.tensor.matmul(out=yps[:, 0:cols], lhsT=wyl, rhs=mov,
                                     start=True, stop=True)
                    xsb = ckpool.tile([P, 480], fp32, tag="xsb")
                    nc.scalar.copy(out=xsb[:, 0:cols], in_=xps[:, 0:cols])
                    xyt = ckpool.tile([P, 480], fp32, tag="xyt")
                    if mg == 1:
                        nc.vector.tensor_tensor(
                            out=xyt[:, 0:ssz], in0=xsb[:, 0:cols],
                            in1=yps[:, 0:cols], op=alu.mult)
                    else:
                        txy = ckpool.tile([P, 480], fp32, tag="txy")
                        nc.vector.tensor_tensor(
                            out=txy[:, 0:cols], in0=xsb[:, 0:cols],
                            in1=yps[:, 0:cols], op=alu.mult)
                        nc.vector.tensor_reduce(
                            out=xyt[:, 0:ssz],
                            in_=txy[:, 0:cols].rearrange(
                                "p (s m) -> p s m", m=mg),
                            axis=mybir.AxisListType.X, op=alu.add,
                        )
                    nc.sync.dma_start(
                        out=xy_d[:, l * NS + s0:l * NS + s0 + ssz],
                        in_=xyt[:, 0:ssz])

            # extract m=0 plane (stride-10 gather) for the rhoi0 output
            r0t = bigpool.tile([P, NS], bf16, tag="r0t")
            nc.gpsimd.tensor_copy(
                out=r0t[:],
                in_=rhoi_sb[:].rearrange("p (s m) -> p s m", m=M10)[:, :, 0],
            )
            nc.sync.dma_start(out=rhoi0_d[:], in_=r0t[:])

    nc.finalize()
    return nc


# ============================ entry point ============================

def kernel(**inputs):
    from concourse.bass_utils import run_bass_kernel_spmd

    species = np.asarray(inputs["species"], np.int64)
    N_NODES = species.shape[0]
    cores, deg = _partition_and_pack(np.asarray(inputs["edge_src"]), N_NODES)
    maxb = max(len(c["blocks"]) for c in cores)
    nchunk = (maxb + CH - 1) // CH
    B = nchunk * CH
    NS = NSLOT * B

    per_core = _build_host_inputs(inputs, cores, deg, B, nchunk)

    wx = np.empty((P, 3 * NCHAN), np.float32)
    wy = np.empty((P, 3 * NCHAN), np.float32)
    for l, key in enumerate(("W0", "W1", "W2")):
        Wp = _perm_w(inputs[key])
        wx[:, l * NCHAN:(l + 1) * NCHAN] = Wp[:, :NCHAN]
        wy[:, l * NCHAN:(l + 1) * NCHAN] = (
            Wp[:, NCHAN:] / np.sqrt(2 * l + 1.0))
    wx = wx.astype(BF16)
    wy = wy.astype(BF16)

    key = nchunk
    if key not in _COMPILED:
        _COMPILED[key] = _build_program(nchunk)
    nc = _COMPILED[key]

    in_maps = [
        {
            "edf": pc["edf"],
            "senc": pc["senc"],
            "oh": pc["oh"],
            "wx": wx,
            "wy": wy,
        }
        for pc in per_core
    ]
    res = run_bass_kernel_spmd(nc, in_maps, list(range(NCORES)),
                               trace=TRACE)
    global LAST_RESULT
    LAST_RESULT = res

    # ---------------- host assembly ----------------
    st = np.asarray(inputs["species_table"], np.float32)
    out = np.zeros((N_NODES, N_SPEC + NB + 3 * NCHAN), np.float32)
    out[:, :N_SPEC] = st[species]

    # device basis row of original index rs = r*16+s is dev = s*8+r
    r = np.arange(NB) // N_SPEC
    s = np.arange(NB) % N_SPEC
    dev_of_rs = s * N_RADIAL + r

    for c in range(NCORES):
        sn = per_core[c]["slot_node"]
        valid = sn >= 0
        nodes = sn[valid]
        slots = np.nonzero(valid)[0]
        r0 = np.asarray(res.results[c]["rhoi0"], np.float32)  # [128, NS]
        xy = res.results[c]["xy"]  # [128, 3*NS]
        out[nodes, N_SPEC:N_SPEC + NB] = r0[dev_of_rs][:, slots].T
        for l in range(3):
            out[nodes, N_SPEC + NB + l * NCHAN:N_SPEC + NB + (l + 1) * NCHAN] = (
                xy[:, l * NS + slots].T)
    return out
